# revision 2
# baseline (speedup 1.0000x reference)
"""Trainium2 Bass kernel for nn_BEM_50002009260181.

Module (B=4, L=1024, D=768, F=32):
    AKey   = tanh(A @ W_aup1.T + b_aup1)          (B,L,D)
    AValue = tan (A @ W_aup2.T + b_aup2)          (B,L,D)
    VKey   = tanh(V @ W_vup1.T + b_vup1)          (B,L,D)
    VValue = tanh(V @ W_vup2.T + b_vup2)          (B,L,D)
    TAQ    = tanh(T * (A @ w_a.T) + b_a)          (B,L,D)
    TVQ    = tanh(T * (V @ w_v.T) + b_v)          (B,L,D)
    ta     = softmax_L(sum_d TAQ*VKey)            (B,L)
    tv     = softmax_L(sum_d TVQ*AKey)            (B,L)
    out    = (AValue * ta[...,None], VValue * tv[...,None])

Sharding: 8 cores = (batch b, L-half h).  Each core computes the full-L
scores for its batch (duplicated across the 2 cores of a batch, avoiding
any cross-core communication for the softmax) and the outputs for its own
L-half.  Inputs are rotated per-core so the own half is always tiles 0-3.
"""

import numpy as np

B, L, D, F = 4, 1024, 768, 32
NCORES = 8
LT = 128          # l-tile size (partition dim)
NT = L // LT      # 8 l-tiles per batch
NT_HALF = NT // 2 # 4 own tiles

PI = float(np.pi)
PIO2_HI = float(np.float32(np.pi / 2))
PIO2_LO = float(np.float64(np.pi / 2) - np.float64(np.float32(np.pi / 2)))

_CACHE = {}


def _build():
    if "nc" in _CACHE:
        return _CACHE["nc"]

    import concourse.bacc as bacc
    import concourse.tile as tile
    import concourse.mybir as mybir

    F32 = mybir.dt.float32
    AF = mybir.ActivationFunctionType
    ALU = mybir.AluOpType

    nc = bacc.Bacc()

    # ---- DRAM I/O (per-core shapes) ----
    d_t = nc.dram_tensor("t_rot", [L, D], F32, kind="ExternalInput")
    d_a = nc.dram_tensor("a_aug", [F + 1, L], F32, kind="ExternalInput")
    d_v = nc.dram_tensor("v_aug", [F + 1, L], F32, kind="ExternalInput")
    d_ra1 = nc.dram_tensor("rhs_a1", [F + 1, D], F32, kind="ExternalInput")
    d_ra2 = nc.dram_tensor("rhs_a2", [F + 1, D], F32, kind="ExternalInput")
    d_rv1 = nc.dram_tensor("rhs_v1", [F + 1, D], F32, kind="ExternalInput")
    d_rv2 = nc.dram_tensor("rhs_v2", [F + 1, D], F32, kind="ExternalInput")
    d_wa = nc.dram_tensor("wa_t", [F, 1], F32, kind="ExternalInput")
    d_wv = nc.dram_tensor("wv_t", [F, 1], F32, kind="ExternalInput")
    d_ba = nc.dram_tensor("ba_vec", [LT, 1], F32, kind="ExternalInput")
    d_bv = nc.dram_tensor("bv_vec", [LT, 1], F32, kind="ExternalInput")
    d_oa = nc.dram_tensor("out_a", [L // 2, D], F32, kind="ExternalOutput")
    d_ov = nc.dram_tensor("out_v", [L // 2, D], F32, kind="ExternalOutput")

    with tile.TileContext(nc) as tc:
        with (
            tc.tile_pool(name="consts", bufs=1) as consts,
            tc.tile_pool(name="tpool", bufs=3) as tpool,
            tc.tile_pool(name="keys", bufs=2) as keys,
            tc.tile_pool(name="vals", bufs=2) as vals,
            tc.tile_pool(name="ps", bufs=4, space="PSUM") as ps,
        ):
            # ---- constants / weights in SBUF ----
            sb_a = consts.tile([F + 1, L], F32, tag="sb_a")
            nc.sync.dma_start(out=sb_a[:], in_=d_a[:])
            sb_v = consts.tile([F + 1, L], F32, tag="sb_v")
            nc.sync.dma_start(out=sb_v[:], in_=d_v[:])
            sb_ra1 = consts.tile([F + 1, D], F32, tag="sb_ra1")
            nc.sync.dma_start(out=sb_ra1[:], in_=d_ra1[:])
            sb_ra2 = consts.tile([F + 1, D], F32, tag="sb_ra2")
            nc.sync.dma_start(out=sb_ra2[:], in_=d_ra2[:])
            sb_rv1 = consts.tile([F + 1, D], F32, tag="sb_rv1")
            nc.sync.dma_start(out=sb_rv1[:], in_=d_rv1[:])
            sb_rv2 = consts.tile([F + 1, D], F32, tag="sb_rv2")
            nc.sync.dma_start(out=sb_rv2[:], in_=d_rv2[:])
            sb_wa = consts.tile([F, 1], F32, tag="sb_wa")
            nc.sync.dma_start(out=sb_wa[:], in_=d_wa[:])
            sb_wv = consts.tile([F, 1], F32, tag="sb_wv")
            nc.sync.dma_start(out=sb_wv[:], in_=d_wv[:])
            sb_ba = consts.tile([LT, 1], F32, tag="sb_ba")
            nc.sync.dma_start(out=sb_ba[:], in_=d_ba[:])
            sb_bv = consts.tile([LT, 1], F32, tag="sb_bv")
            nc.sync.dma_start(out=sb_bv[:], in_=d_bv[:])

            sb_lo = consts.tile([LT, 1], F32, tag="sb_lo")
            nc.vector.memset(sb_lo[:], PIO2_LO)
            ones_col = consts.tile([LT, 1], F32, tag="ones_col")
            nc.vector.memset(ones_col[:], 1.0)
            ones_row = consts.tile([1, LT], F32, tag="ones_row")
            nc.vector.memset(ones_row[:], 1.0)

            # ---- qa/qv: per-l scalars via tiny matmuls ----
            ps_q = ps.tile([LT, 2 * NT], F32, tag="ps")
            for i in range(NT):
                nc.tensor.matmul(
                    ps_q[:, i : i + 1],
                    sb_a[:F, i * LT : (i + 1) * LT],
                    sb_wa[:],
                    start=True, stop=True,
                )
                nc.tensor.matmul(
                    ps_q[:, NT + i : NT + i + 1],
                    sb_v[:F, i * LT : (i + 1) * LT],
                    sb_wv[:],
                    start=True, stop=True,
                )
            sb_q = consts.tile([LT, 2 * NT], F32, tag="sb_q")
            nc.scalar.copy(out=sb_q[:], in_=ps_q[:])

            s_ta = consts.tile([LT, NT], F32, tag="s_ta")
            s_tv = consts.tile([LT, NT], F32, tag="s_tv")

            # ---- score phase: full L ----
            for i in range(NT):
                lsl = slice(i * LT, (i + 1) * LT)
                t_t = tpool.tile([LT, D], F32, tag="t")
                nc.sync.dma_start(out=t_t[:], in_=d_t[lsl, :])

                ps_ak = ps.tile([LT, D], F32, tag="ps")
                nc.tensor.matmul(ps_ak[:, 0:512], sb_a[:, lsl], sb_ra1[:, 0:512], start=True, stop=True)
                nc.tensor.matmul(ps_ak[:, 512:D], sb_a[:, lsl], sb_ra1[:, 512:D], start=True, stop=True)
                ps_vk = ps.tile([LT, D], F32, tag="ps")
                nc.tensor.matmul(ps_vk[:, 0:512], sb_v[:, lsl], sb_rv1[:, 0:512], start=True, stop=True)
                nc.tensor.matmul(ps_vk[:, 512:D], sb_v[:, lsl], sb_rv1[:, 512:D], start=True, stop=True)

                akey = keys.tile([LT, D], F32, tag="akey")
                nc.scalar.activation(out=akey[:], in_=ps_ak[:], func=AF.Tanh)
                vkey = keys.tile([LT, D], F32, tag="vkey")
                nc.scalar.activation(out=vkey[:], in_=ps_vk[:], func=AF.Tanh)
                taq = keys.tile([LT, D], F32, tag="taq")
                nc.scalar.activation(out=taq[:], in_=t_t[:], func=AF.Tanh,
                                     bias=sb_ba[:], scale=sb_q[:, i : i + 1])
                tvq = keys.tile([LT, D], F32, tag="tvq")
                nc.scalar.activation(out=tvq[:], in_=t_t[:], func=AF.Tanh,
                                     bias=sb_bv[:], scale=sb_q[:, NT + i : NT + i + 1])

                scr = keys.tile([LT, D], F32, tag="scr")
                nc.vector.scalar_tensor_tensor(
                    out=scr[:], in0=taq[:], scalar=1.0, in1=vkey[:],
                    op0=ALU.mult, op1=ALU.mult, accum_out=s_ta[:, i : i + 1],
                )
                scr2 = keys.tile([LT, D], F32, tag="scr2")
                nc.vector.scalar_tensor_tensor(
                    out=scr2[:], in0=tvq[:], scalar=1.0, in1=akey[:],
                    op0=ALU.mult, op1=ALU.mult, accum_out=s_tv[:, i : i + 1],
                )

            # ---- softmax over all 1024 l's (no max subtraction; |s| < 40) ----
            e_ta = consts.tile([LT, NT], F32, tag="e_ta")
            e_tv = consts.tile([LT, NT], F32, tag="e_tv")
            rsum = consts.tile([LT, 2], F32, tag="rsum")
            nc.scalar.activation(out=e_ta[:], in_=s_ta[:], func=AF.Exp, accum_out=rsum[:, 0:1])
            nc.scalar.activation(out=e_tv[:], in_=s_tv[:], func=AF.Exp, accum_out=rsum[:, 1:2])
            ps_z = ps.tile([1, 2], F32, tag="ps")
            nc.tensor.matmul(ps_z[:], ones_col[:], rsum[:], start=True, stop=True)
            invz = consts.tile([1, 2], F32, tag="invz")
            nc.vector.reciprocal(out=invz[:], in_=ps_z[:])
            ps_b = ps.tile([LT, 2], F32, tag="ps")
            nc.tensor.matmul(ps_b[:], ones_row[:], invz[:], start=True, stop=True)
            invzb = consts.tile([LT, 2], F32, tag="invzb")
            nc.scalar.copy(out=invzb[:], in_=ps_b[:])
            ta_n = consts.tile([LT, NT_HALF], F32, tag="ta_n")
            nc.vector.tensor_scalar(out=ta_n[:], in0=e_ta[:, 0:NT_HALF],
                                    scalar1=invzb[:, 0:1], scalar2=None, op0=ALU.mult)
            tv_n = consts.tile([LT, NT_HALF], F32, tag="tv_n")
            nc.vector.tensor_scalar(out=tv_n[:], in0=e_tv[:, 0:NT_HALF],
                                    scalar1=invzb[:, 1:2], scalar2=None, op0=ALU.mult)

            # ---- value phase: own half (tiles 0..3) ----
            # VValue (tanh) first so ACT stays on the tanh/exp table set,
            # then all sin/cos (single table switch).
            ps_xvs = []
            for j in range(NT_HALF):
                lsl = slice(j * LT, (j + 1) * LT)
                ps_xv = ps.tile([LT, D], F32, tag="ps")
                nc.tensor.matmul(ps_xv[:, 0:512], sb_v[:, lsl], sb_rv2[:, 0:512], start=True, stop=True)
                nc.tensor.matmul(ps_xv[:, 512:D], sb_v[:, lsl], sb_rv2[:, 512:D], start=True, stop=True)
                vval = vals.tile([LT, D], F32, tag="vval")
                nc.scalar.activation(out=vval[:], in_=ps_xv[:], func=AF.Tanh)
                outv = vals.tile([LT, D], F32, tag="outv")
                nc.vector.tensor_scalar(out=outv[:], in0=vval[:],
                                        scalar1=tv_n[:, j : j + 1], scalar2=None, op0=ALU.mult)
                nc.sync.dma_start(out=d_ov[lsl, :], in_=outv[:])

            for j in range(NT_HALF):
                lsl = slice(j * LT, (j + 1) * LT)
                ps_xa = ps.tile([LT, D], F32, tag="ps")
                nc.tensor.matmul(ps_xa[:, 0:512], sb_a[:, lsl], sb_ra2[:, 0:512], start=True, stop=True)
                nc.tensor.matmul(ps_xa[:, 512:D], sb_a[:, lsl], sb_ra2[:, 512:D], start=True, stop=True)

                # sin(x) via range-wrap into [-pi, pi]
                rs = vals.tile([LT, D], F32, tag="rs")
                nc.vector.add_range_wrap(out=rs[:], in_=ps_xa[:], shift=0.0, bound=PI, period=2 * PI)
                sn = vals.tile([LT, D], F32, tag="sn")
                nc.scalar.activation(out=sn[:], in_=rs[:], func=AF.Sin)
                # cos(x) accurately (Cody-Waite around the tan pole):
                # cos(x) = cos(rs) (wrap adds exactly 0 near the pole), and
                # rr = pi/2_hi - |rs| is exact near the pole (Sterbenz);
                # cos = Sin(rr + pi/2_lo).
                nax = vals.tile([LT, D], F32, tag="nax")
                nc.vector.scalar_tensor_tensor(
                    out=nax[:], in0=rs[:], scalar=-1.0, in1=rs[:],
                    op0=ALU.mult, op1=ALU.min,
                )
                rr = vals.tile([LT, D], F32, tag="rr")
                nc.vector.tensor_scalar(out=rr[:], in0=nax[:], scalar1=PIO2_HI,
                                        scalar2=None, op0=ALU.add)
                cs = vals.tile([LT, D], F32, tag="cs")
                nc.scalar.activation(out=cs[:], in_=rr[:], func=AF.Sin, bias=sb_lo[:])
                rc = vals.tile([LT, D], F32, tag="rc")
                nc.vector.reciprocal_approx_fast(out=rc[:], in_=cs[:])
                outa = vals.tile([LT, D], F32, tag="outa")
                nc.vector.scalar_tensor_tensor(
                    out=outa[:], in0=sn[:], scalar=ta_n[:, j : j + 1], in1=rc[:],
                    op0=ALU.mult, op1=ALU.mult,
                )
                nc.sync.dma_start(out=d_oa[lsl, :], in_=outa[:])

    nc.finalize()
    _CACHE["nc"] = nc
    return nc


def _prep_in_maps(T, A, V, w_a, b_a, w_v, b_v,
                  W_aup1, b_aup1, W_aup2, b_aup2,
                  W_vup1, b_vup1, W_vup2, b_vup2):
    f32 = np.float32
    T = np.ascontiguousarray(np.asarray(T, f32))
    A = np.asarray(A, f32)
    V = np.asarray(V, f32)

    def aug_w(W, b):
        return np.ascontiguousarray(
            np.concatenate([np.asarray(W, f32).T, np.asarray(b, f32)[None, :]], axis=0)
        )

    rhs_a1 = aug_w(W_aup1, b_aup1)
    rhs_a2 = aug_w(W_aup2, b_aup2)
    rhs_v1 = aug_w(W_vup1, b_vup1)
    rhs_v2 = aug_w(W_vup2, b_vup2)
    wa_t = np.ascontiguousarray(np.asarray(w_a, f32).reshape(1, F).T)
    wv_t = np.ascontiguousarray(np.asarray(w_v, f32).reshape(1, F).T)
    ba_vec = np.full((LT, 1), np.asarray(b_a, f32).reshape(()), f32)
    bv_vec = np.full((LT, 1), np.asarray(b_v, f32).reshape(()), f32)

    in_maps = []
    for c in range(NCORES):
        b, h = divmod(c, 2)
        rot = np.r_[np.arange(512 * h, L), np.arange(0, 512 * h)]
        a_aug = np.concatenate([A[b].T, np.ones((1, L), f32)], axis=0)[:, rot]
        v_aug = np.concatenate([V[b].T, np.ones((1, L), f32)], axis=0)[:, rot]
        in_maps.append({
            "t_rot": np.ascontiguousarray(T[b][rot]),
            "a_aug": np.ascontiguousarray(a_aug),
            "v_aug": np.ascontiguousarray(v_aug),
            "rhs_a1": rhs_a1, "rhs_a2": rhs_a2,
            "rhs_v1": rhs_v1, "rhs_v2": rhs_v2,
            "wa_t": wa_t, "wv_t": wv_t,
            "ba_vec": ba_vec, "bv_vec": bv_vec,
        })
    return in_maps


def kernel(**inputs):
    from concourse.bass_utils import run_bass_kernel_spmd

    nc = _build()
    in_maps = _prep_in_maps(**inputs)
    res = run_bass_kernel_spmd(nc, in_maps, core_ids=list(range(NCORES)))

    out_a = np.empty((B, L, D), np.float32)
    out_v = np.empty((B, L, D), np.float32)
    for c in range(NCORES):
        b, h = divmod(c, 2)
        out_a[b, 512 * h : 512 * (h + 1)] = res.results[c]["out_a"]
        out_v[b, 512 * h : 512 * (h + 1)] = res.results[c]["out_v"]
    return out_a, out_v
